# revision 64
# baseline (speedup 1.0000x reference)
"""Trainium2 Bass kernel for nn_AlibiBlock (dense transformer block with ALiBi).

Contract: kernel(**inputs) takes the FULL unsharded inputs (numpy or jax,
shapes from setup_inputs) and returns the FULL [2, 2048, 1024] float32 output.

Sharding (8 NeuronCores = 2 groups of 4):
  - data parallel over batch (B=2): cores 0-3 <- batch 0, cores 4-7 <- batch 1
  - tensor parallel over heads inside each group for attention (16 heads -> 4
    per core); one grouped ReduceScatter per query group hands each core the
    summed residual update for its OWN T-slice.
  - the MLP runs T-parallel (each core: its T-quarter with the FULL 4096
    hidden, weights streamed), split by query group so each half overlaps a
    ReduceScatter of the other half.

v4 structure (vs v2 baseline):
  - startup: bias consts load as single batched [P,n] DMAs (v2 issued 54
    per-column DMAs, serializing ~50us of sync-queue issue ahead of the x
    tiles); warm-collective DMAs moved to the gpsimd queue; mask/alibi on
    the scalar queue; the 8MB resident wfc2 load is emitted after the x
    tiles in sync-queue order.
  - qkv and proj run as fp8e4m3 DoubleRow matmuls (weights pre-scaled by
    WS=64 on the host, descaled by 1/WS in the evictions; h^T and the
    divided attention output yd are stored fp8). Measured rel err ~6.8e-3
    vs the 2e-2 gate (fp8 for the MLP matmuls was tried and rejected: each
    fp8 tensor-point there costs ~1.2e-2).
  - proj is computed token-major (out[token, C]: yd is the stationary
    operand, wproj the moving one), so the ReduceScatter buffers are
    [rank, TS, C], each proj strip is ONE contiguous DMA, the post-RS
    residual needs NO PE transposes, and b_proj folds into the host-side
    xq copy.
  - fc2 is also token-major: psum[token, C] accumulates fcT^T @ wfc2 over
    the 32 hidden k-tiles plus a K=1 ones-row matmul adding b_fc2; the
    eviction fuses the x2 residual add and streams straight out. wfc2 is
    SBUF-resident bf16 (loaded once); wfc streams in 1MB chunks.
  - group 1's fc (the post-RS1 tail) runs fp8 DoubleRow from a second,
    WS-prescaled fp8 copy of wfc (h2T(g1) stored fp8); group 0's fc stays
    bf16 since it is woven under attention g1 for free. Measured rel err
    1.39e-2. fp8 RS wire was tried: same speed, worse margin - rejected.
  - x is passed pre-cast to bf16 (LN1 is its only consumer; the residual
    path uses the fp32 xq), halving the startup DMA and doubling the DVE
    rate of the LN1 stats/apply; the x2 residual tiles are bf16 too, and
    the freed SBUF deepens the xs/ptp pipelines. Final: 485972 ns at rel
    err 1.40e-2.

Per-core dataflow (T=2048, C=1024, 4 heads of d=64, fp32 PSUM
accumulation, bf16 residual stream):
  LN1 via bn_stats in [T,C] layout; PE-transpose -> h^T [C,T] fp8;
  qkv^T = Wqkv^T @ h^T (DoubleRow, 1/WS+bias fused in DVE eviction), LN
  tiles 8-15 woven between the qkv column chunks; v transposed into vaug
  with a ones column (row 64 of the PV psum = softmax denominator);
  per query group and head: S^T tile = k^T @ q^T narrowed to columns >=
  kt*P-qcol; P^T = Exp(S/sqrt(d) - slope*k) in one narrowed ACT op;
  causal mask = bf16 multiply on the [128,128] diagonal block only; PV
  accumulates V_aug^T @ P^T; denominator divide via reciprocal + PE
  ones-row broadcast + DVE multiply into yd fp8;
  proj (token-major DoubleRow) -> grouped ReduceScatter (bf16);
  then per query group: residual + LN2 -> h2^T -> fc (gelu+bias in ACT
  eviction) -> token-major fc2 + residual -> out slice; the g=0 MLP is
  woven under attention g1 / RS_0, fc2(g0) overlaps RS_1.

LN affine params are folded into the qkv/fc weights on the host.
"""

import math
import sys

for _p in ("/opt/trn_rl_repo",):
    if _p not in sys.path:
        sys.path.insert(0, _p)

import numpy as np
import ml_dtypes

import concourse.bass as bass
import concourse.mybir as mybir
import concourse.tile as tile
from concourse import bacc
from concourse.bass_utils import run_bass_kernel_spmd
from concourse.masks import make_identity

BF16 = mybir.dt.bfloat16
FP8 = mybir.dt.float8e4
F32 = mybir.dt.float32
AF = mybir.ActivationFunctionType
DR = mybir.MatmulPerfMode.DoubleRow
WS = 64.0            # fp8 weight pre-scale for the MLP weights

C = 1024            # model dim
NH_LOC = 4          # heads per core
D = 64              # head dim
EPS = 1e-5
NCORES = 8
GROUPS = [[0, 1, 2, 3], [4, 5, 6, 7]]
P = 128
QTW = 512           # matmul free-dim tile (one PSUM bank)
QG = 1024           # query group / pipeline chunk width


def _build(T: int):
    """Build + compile the SPMD program for sequence length T (multiple of QG)."""
    TPT = T // P        # token partition-tiles
    CT = C // P         # 8
    NQG = T // QG       # query-group chunks
    TQ = T // 4         # T-quarter owned by each core after ReduceScatter
    QTT = TQ // P       # local token tiles
    TS = QG // 4        # local tokens contributed by one query group
    QTTg = TS // P      # local token tiles per query group
    FT = 4 * C // P     # 32 hidden partition-tiles (full MLP hidden)

    nc = bacc.Bacc("TRN2", target_bir_lowering=False, debug=False,
                   num_devices=NCORES)

    x_d = nc.dram_tensor("x", [T, C], BF16, kind="ExternalInput")
    xq_d = nc.dram_tensor("xq", [TQ, C], F32, kind="ExternalInput")
    wqkv_d = nc.dram_tensor("wqkv", [C, 3 * NH_LOC * D], FP8, kind="ExternalInput")
    bqkv_d = nc.dram_tensor("bqkv", [P, 6], F32, kind="ExternalInput")
    wproj_d = nc.dram_tensor("wproj", [NH_LOC * D, C], FP8, kind="ExternalInput")
    wfc_d = nc.dram_tensor("wfc", [C, 4 * C], BF16, kind="ExternalInput")
    wfc8_d = nc.dram_tensor("wfc8", [C, 4 * C], FP8, kind="ExternalInput")
    bfc_d = nc.dram_tensor("bfc", [P, FT], F32, kind="ExternalInput")
    wfc2_d = nc.dram_tensor("wfc2", [4 * C, C], BF16, kind="ExternalInput")
    bfc2_d = nc.dram_tensor("bfc2r", [1, C], BF16, kind="ExternalInput")
    alibi_d = nc.dram_tensor("alibi", [P, NH_LOC * TPT], F32, kind="ExternalInput")
    mask_d = nc.dram_tensor("mask", [P, P], BF16, kind="ExternalInput")
    out_d = nc.dram_tensor("out", [TQ, C], F32, kind="ExternalOutput")

    x_t = x_d.ap().rearrange("(n p) c -> n p c", p=P)
    xq_t = xq_d.ap().rearrange("(n p) c -> n p c", p=P)
    out_t = out_d.ap().rearrange("(n p) c -> n p c", p=P)
    wqkv_r = wqkv_d.ap().rearrange("(k p) m -> p k m", p=P)  # [P, CT, 768]
    wproj_r = wproj_d.ap().rearrange("(k p) m -> p k m", p=P)  # [P, 2, C]
    wfc_r = wfc_d.ap().rearrange("(k p) m -> p k m", p=P)    # [P, CT, 4C]
    wfc8_r = wfc8_d.ap().rearrange("(k p) m -> p k m", p=P)  # [P, CT, 4C] fp8
    wfc2_r = wfc2_d.ap().rearrange("(k p) m -> p k m", p=P)  # [P, FT, C]

    import contextlib

    with tile.TileContext(nc) as tc, contextlib.ExitStack() as es:
        const = es.enter_context(tc.tile_pool(name="const", bufs=1))
        wproj_pool = es.enter_context(tc.tile_pool(name="wprojp", bufs=1))
        dram = es.enter_context(tc.tile_pool(name="dram", bufs=1, space="DRAM"))
        psum = es.enter_context(tc.tile_pool(name="psum", bufs=2, space="PSUM"))
        xs = es.enter_context(tc.tile_pool(name="xs", bufs=3))
        hp = es.enter_context(tc.tile_pool(name="hp", bufs=3))
        lnp = es.enter_context(tc.tile_pool(name="lnp", bufs=6))
        strip = es.enter_context(tc.tile_pool(name="strip", bufs=2))
        xo = es.enter_context(tc.tile_pool(name="xo", bufs=1))
        apool = es.enter_context(tc.tile_pool(name="apool", bufs=1))
        ptp = es.enter_context(tc.tile_pool(name="ptp", bufs=5))
        rbp = es.enter_context(tc.tile_pool(name="rbp", bufs=1))

        # ---- constants ----
        # small consts go first on the sync queue as SINGLE batched DMAs (the
        # v2 kernel issued 54 per-column DMAs here, serializing ~50us of sync
        # queue issue ahead of the first x tile); the big resident MLP weights
        # stream on the scalar/gpsimd queues so they never block the x tiles.
        ident = const.tile([P, P], BF16)
        make_identity(nc, ident)
        mask_sb = const.tile([P, P], BF16)
        nc.scalar.dma_start(out=mask_sb[:], in_=mask_d.ap())
        alibi_sb = const.tile([P, NH_LOC * TPT], F32)
        nc.scalar.dma_start(out=alibi_sb[:], in_=alibi_d.ap())
        eps_sb = const.tile([P, 1], F32)
        nc.vector.memset(eps_sb[:], EPS)
        ones64 = const.tile([1, D], F32)
        nc.vector.memset(ones64[:], 1.0)
        rws_sb = const.tile([P, 1], F32)
        nc.vector.memset(rws_sb[:], 1.0 / WS)
        ones1 = const.tile([1, P], BF16)
        nc.vector.memset(ones1[:], 1.0)
        bqkv_sb = const.tile([P, 6], F32)
        nc.sync.dma_start(out=bqkv_sb[:], in_=bqkv_d.ap())
        bfc_sb = const.tile([P, FT], F32)
        nc.scalar.dma_start(out=bfc_sb[:], in_=bfc_d.ap())
        bfc2row = const.tile([1, C], BF16)
        nc.scalar.dma_start(out=bfc2row[:], in_=bfc2_d.ap())

        # fp8 DoubleRow attention weights (pre-scaled by WS on the host)
        wproj8 = const.tile([P, 2, C], FP8)
        nc.scalar.dma_start(out=wproj8[:], in_=wproj_r[:])

        # resident bf16 fc2 weights [P, ktile, C]; the DMA is emitted late
        # (after the LN1 x tiles) so its 8MB never starves the x loads
        wfc2sb = const.tile([P, FT, C], BF16)

        def load_wfc2():
            for h in range(2):
                nc.sync.dma_start(out=wfc2sb[:, 16 * h:16 * (h + 1), :],
                                  in_=wfc2_r[:, 16 * h:16 * (h + 1), :])

        # Per-query-group ReduceScatter buffers; each core owns rank-slice
        # cols [r*TS, (r+1)*TS) of every query group.
        # token-major proj partials: shard r = rank r's token slice [TS, C]
        rs_in = [dram.tile([4, TS, C], BF16, name=f"rs_in{g}")
                 for g in range(NQG)]
        rs_out = [dram.tile([TS, C], BF16, name=f"rs_out{g}")
                  for g in range(NQG)]

        warm_in = dram.tile([4, 1, P], BF16, name="warm_in")
        warm_out = dram.tile([1, P], BF16, name="warm_out")
        zrow = const.tile([1, P], BF16)
        nc.gpsimd.memset(zrow[:], 0.0)
        for r in range(4):
            nc.gpsimd.dma_start(out=warm_in[r], in_=zrow[:])
        nc.gpsimd.collective_compute(
            "ReduceScatter", mybir.AluOpType.add, replica_groups=GROUPS,
            ins=[warm_in.opt()], outs=[warm_out.opt()])

        def layernorm_tile(x_tile, h_tile):
            """h = (x - mean)/sqrt(var+eps), bf16 out. x [P, C] fp32."""
            st = lnp.tile([P, 2, 6], F32, name="st")
            xr = x_tile.rearrange("p (a b) -> p a b", a=2)
            for a in range(2):
                nc.vector.bn_stats(out=st[:, a, :], in_=xr[:, a, :])
            mv = lnp.tile([P, 2], F32, name="mv")
            nc.vector.bn_aggr(out=mv[:], in_=st[:])
            rs = lnp.tile([P, 1], F32, name="rs")
            nc.scalar.activation(out=rs[:], in_=mv[:, 1:2], func=AF.Sqrt,
                                 bias=eps_sb[:], scale=1.0)
            nc.vector.reciprocal(out=rs[:], in_=rs[:])
            nc.vector.tensor_scalar(out=h_tile[:], in0=x_tile[:],
                                    scalar1=mv[:, 0:1], scalar2=rs[:],
                                    op0=mybir.AluOpType.subtract,
                                    op1=mybir.AluOpType.mult)

        def transpose_grouped(h_tile, dst, tag="ps", on_act=False):
            """h [P(tok), C] -> dst [P, CT, P] column block (one evict)."""
            tp = psum.tile([P, CT, P], BF16, tag=tag,
                           bufs=(1 if tag == "fill" else None), name="tp")
            for j in range(CT):
                nc.tensor.transpose(tp[:, j, :], h_tile[:, j * P:(j + 1) * P],
                                    ident[:])
            if on_act:
                nc.scalar.activation(out=dst, in_=tp[:], func=AF.Copy)
            else:
                nc.vector.tensor_copy(out=dst, in_=tp[:])

        # ---- scoped pools: qkv weights + h^T live only through qkv ----
        es_qkv = contextlib.ExitStack()
        wqkv_pool = es_qkv.enter_context(tc.tile_pool(name="wqkvp", bufs=1))
        hTp = es_qkv.enter_context(tc.tile_pool(name="hTp", bufs=1))
        wqkv8 = wqkv_pool.tile([P, CT, 3 * NH_LOC * D], FP8, name="wqkv8")
        nc.sync.dma_start(out=wqkv8[:], in_=wqkv_r[:])
        hT = hTp.tile([P, CT, T], FP8, name="hT")

        qkvT = apool.tile([P, 6, T], BF16, name="qkvT")
        vaug = apool.tile([P, NH_LOC, TPT, D + 1], BF16, name="vaug")
        yd = apool.tile([P, 2, T], FP8, name="yd")

        def ln_tile(tt):
            # x arrives pre-cast to bf16 (LN1 is the only consumer of the
            # full x; the residual path uses the fp32 xq) -> half the DMA
            # bytes and 2x DVE rate on bn_stats/apply; hT evict on ACT
            x_tile = xs.tile([P, C], BF16, tag="xs", name="xsb")
            nc.sync.dma_start(out=x_tile[:], in_=x_t[tt])
            h_tile = hp.tile([P, C], BF16, name="hp")
            layernorm_tile(x_tile, h_tile)
            transpose_grouped(h_tile, hT[:, :, tt * P:(tt + 1) * P],
                              on_act=True)

        def gen_qkv_m(g, m, tag="ps"):
            """generator: one qkv output strip (fp8 DoubleRow); yields every
            256-col sub-block so it can weave between attention kts."""
            ps = psum.tile([P, 2, QTW], F32, tag=tag,
                           bufs=(1 if tag == "fill" else None), name="ps")
            for half in range(2):
                col = g * QG + half * QTW
                for c2 in range(2):
                    for k2 in range(CT // 2):
                        nc.tensor.matmul(
                            ps[:, half, c2 * 256:(c2 + 1) * 256],
                            wqkv8[:, 2 * k2:2 * k2 + 2, m * P:(m + 1) * P],
                            hT[:, 2 * k2:2 * k2 + 2,
                               col + c2 * 256:col + (c2 + 1) * 256],
                            start=(k2 == 0), stop=(k2 == CT // 2 - 1),
                            perf_mode=DR)
                    yield
            nc.vector.tensor_scalar(
                out=qkvT[:, m, g * QG:(g + 1) * QG],
                in0=ps.rearrange("p a b -> p (a b)"),
                scalar1=rws_sb[:], scalar2=bqkv_sb[:, m:m + 1],
                op0=mybir.AluOpType.mult, op1=mybir.AluOpType.add)

        def gen_vaug(g, hs=None, tag="ps"):
            kts = range(g * 8, min((g + 1) * 8, TPT))
            for h in (range(NH_LOC) if hs is None else hs):
                voff = (h % 2) * D
                tpv = psum.tile([P, len(kts), D], BF16, tag=tag,
                                bufs=(1 if tag == "fill" else None), name="tpv")
                for i, kt in enumerate(kts):
                    nc.tensor.transpose(
                        tpv[:, i, :],
                        qkvT[voff:voff + D, 4 + h // 2, kt * P:(kt + 1) * P],
                        ident[voff:voff + D, voff:voff + D])
                nc.vector.tensor_copy(out=vaug[:, h, kts.start:kts.stop, 0:D],
                                      in_=tpv[:])
                nc.vector.memset(vaug[:, h, kts.start:kts.stop, D:D + 1], 1.0)
                yield

        def run_gen(gn):
            for _ in gn:
                pass

        def filler_steps(gens):
            """Flatten generators into a step-callable for attention weaving.
            Operates on the caller's list IN PLACE so work appended later is
            picked up."""

            def step(n=1):
                for _ in range(n):
                    while gens:
                        try:
                            next(gens[0])
                            break
                        except StopIteration:
                            gens.pop(0)

            def drain():
                while gens:
                    run_gen(gens.pop(0))

            step.drain = drain
            return step

        # ---------- LN1 + qkv + vaug ----------
        # qkv(0) strip steps weave between the LN tiles so the PE is dense
        # from the start: half0 steps (token tiles 0-3) may run from tile 4,
        # half1 steps (tiles 4-7) from tile 8.
        def gen_qkv_half(m, half):
            """one 512-wide half of a g0 qkv strip (fp8 DoubleRow); own
            1-bank psum + per-half eviction so the fill slot releases."""
            ps = psum.tile([P, QTW], F32, tag="fill", bufs=1, name="psh")
            col = half * QTW
            for c2 in range(2):
                if c2 == 1:
                    yield
                for k2 in range(CT // 2):
                    nc.tensor.matmul(
                        ps[:, c2 * 256:(c2 + 1) * 256],
                        wqkv8[:, 2 * k2:2 * k2 + 2, m * P:(m + 1) * P],
                        hT[:, 2 * k2:2 * k2 + 2,
                           col + c2 * 256:col + (c2 + 1) * 256],
                        start=(k2 == 0), stop=(k2 == CT // 2 - 1),
                        perf_mode=DR)
            nc.vector.tensor_scalar(
                out=qkvT[:, m, col:col + QTW], in0=ps[:],
                scalar1=rws_sb[:], scalar2=bqkv_sb[:, m:m + 1],
                op0=mybir.AluOpType.mult, op1=mybir.AluOpType.add)

        a_ms = (0, 2, 4) if NQG == 2 else tuple(range(6))
        if NQG == 2 and TPT == 16:
            # halves sequenced so half1 (token tiles 4-7) only emits after
            # ln_tile(7); each half-gen fully releases before the next starts
            order = [gen_qkv_half(m, h) for h, pair in
                     ((0, (0, 2)), (1, (0, 2)), (0, (4,)), (1, (4,)))
                     for m in pair]
            seq = [gn for gn in order for _ in range(2)]
            oi = 0
            for tt in range(TPT):
                ln_tile(tt)
                if tt >= 4 and oi < len(seq):
                    next(seq[oi], None)
                    oi += 1
            while oi < len(seq):
                next(seq[oi], None)
                oi += 1
            for gn in order:
                run_gen(gn)                          # absorb StopIterations
        else:
            for tt in range(TPT):
                ln_tile(tt)
            for m in a_ms:
                run_gen(gen_qkv_m(0, m))
        run_gen(gen_vaug(0, hs=((0, 1) if NQG == 2 else None)))
        load_wfc2()     # 8MB resident load, after the x tiles in queue order

        # mid / MLP pools open after the last qkv use so they can reuse the
        # qkv-weight + h^T SBUF space; variables bound here, filled by
        # open_mlp_pools() at the right program point.
        midp = fcp = wstp = None
        x2q = h2Tq = None
        fcT = {}

        def open_mlp_pools():
            nonlocal midp, fcp, wstp, x2q, h2Tq
            es_qkv.close()
            midp = es.enter_context(tc.tile_pool(name="midp", bufs=1))
            # bufs=1: fcT(g1) writes begin only after fc2(g0) consumed fcT(g0)
            fcp = es.enter_context(tc.tile_pool(name="fcp", bufs=1))
            wstp = es.enter_context(tc.tile_pool(name="wstp", bufs=2))
            x2q = [midp.tile([P, C], BF16, name=f"x2q{t}")
                   for t in range(QTT)]
            # per-group h2^T: g0 bf16 (fc in bf16, woven under attention g1),
            # g1 fp8 (fc in fp8 DoubleRow to shrink the post-RS1 tail)
            h2Tq = [midp.tile([P, CT, TS], BF16, name="h2Tq0"),
                    midp.tile([P, CT, TS], FP8, name="h2Tq1")]
            for g in range(NQG):
                fcT[g] = fcp.tile([P, FT, TS], BF16, tag="fcT", name="fcT")

        def attention_head(g, h, filler=None, prologue=None):
            """One head, kt chains pipelined (scores of kt+1 interleave the PV
            of kt); score, exp and PV narrowed to the causal staircase.
            `filler` is a step-callable popped once per kt so the PE stays
            dense while ACT streams the exps."""
            qcol = g * QG
            nkt = 8 * g + 8
            off = (h % 2) * D
            qT = qkvT[off:off + D, h // 2, :]
            kT = qkvT[off:off + D, 2 + h // 2, :]
            if prologue is not None:
                run_gen(prologue)
            yps = psum.tile([D + 1, 2, QTW], F32, tag="yps", bufs=1,
                            name="yps")

            def emit_pv(kt, r, pt):
                last = (kt == nkt - 1)
                if r < QTW:
                    nc.tensor.matmul(
                        yps[:, 0, r:QTW], vaug[:, h, kt, :], pt[:, r:QTW],
                        start=(kt == 0), stop=(kt == 8 * g + 3))
                    nc.tensor.matmul(
                        yps[:, 1, :], vaug[:, h, kt, :], pt[:, QTW:QG],
                        start=(kt == 0), stop=last)
                else:
                    nc.tensor.matmul(
                        yps[:, 1, r - QTW:], vaug[:, h, kt, :],
                        pt[:, r:QG], start=False, stop=last)

            pending = None
            for kt in range(nkt):
                rr = kt * P - qcol
                r = max(rr, 0)
                bias_ap = alibi_sb[:, h * TPT + kt:h * TPT + kt + 1]
                sps = psum.tile([P, 2, QTW], F32, tag="ps", name="sps")
                if r < QTW:
                    nc.tensor.matmul(
                        sps[:, 0, r:QTW], kT[:, kt * P:(kt + 1) * P],
                        qT[:, qcol + r:qcol + QTW], start=True, stop=True)
                    nc.tensor.matmul(
                        sps[:, 1, :], kT[:, kt * P:(kt + 1) * P],
                        qT[:, qcol + QTW:qcol + QG], start=True, stop=True)
                else:
                    nc.tensor.matmul(
                        sps[:, 1, r - QTW:], kT[:, kt * P:(kt + 1) * P],
                        qT[:, qcol + r:qcol + QG], start=True, stop=True)
                pt = ptp.tile([P, QG], BF16, name="pt")
                nc.scalar.activation(
                    out=pt[:, r:], in_=sps.rearrange("p a b -> p (a b)")[:, r:],
                    func=AF.Exp, bias=bias_ap, scale=1.0 / math.sqrt(D))
                if rr >= 0:
                    nc.vector.tensor_mul(
                        out=pt[:, r:r + P], in0=pt[:, r:r + P],
                        in1=mask_sb[:])
                if pending is not None:
                    emit_pv(*pending)
                pending = (kt, r, pt)
                if filler is not None:
                    filler()
            if pending is not None:
                emit_pv(*pending)

            def gen_division():
                dn = rbp.tile([1, QG], F32, name="dn")
                nc.vector.tensor_copy(
                    out=dn[:],
                    in_=yps[D:D + 1, :, :].rearrange("p a b -> p (a b)"))
                nc.vector.reciprocal_approx_fast(out=dn[:], in_=dn[:])
                # broadcast 1/dn over D partitions on the PE (K=1 matmul with
                # a ones row) so gpsimd stays free for collectives
                rbps = psum.tile([D, 2, QTW], F32, tag="fill", bufs=1,
                                 name="rbps")
                for half in range(2):
                    nc.tensor.matmul(rbps[:, half, :], ones64[:],
                                     dn[:, half * QTW:(half + 1) * QTW],
                                     start=True, stop=True)
                rb = rbp.tile([D, QG], F32, name="rb")
                nc.vector.tensor_copy(
                    out=rb[:], in_=rbps.rearrange("p a b -> p (a b)"))
                nc.vector.tensor_mul(
                    out=yd[off:off + D, h // 2, qcol:qcol + QG],
                    in0=yps[0:D, :, :].rearrange("p a b -> p (a b)"),
                    in1=rb[:])
                yield

            return gen_division()

        def proj_chunk(g):
            """token-major proj (fp8 DoubleRow): out[token, C] per 128-token
            tile; strips go straight into the rank-sliced RS buffer with a
            single contiguous DMA each; b_proj is folded into xq on the
            host, so the eviction is just the 1/WS descale."""
            for tg in range(CT):
                tok = g * QG + tg * P
                ps = psum.tile([P, 2, QTW], F32, tag="ps", name="ps")
                for half in range(2):
                    for c2 in range(2):
                        c0 = half * QTW + c2 * 256
                        nc.tensor.matmul(
                            ps[:, half, c2 * 256:(c2 + 1) * 256],
                            yd[:, :, tok:tok + P],
                            wproj8[:, :, c0:c0 + 256],
                            start=True, stop=True, perf_mode=DR)
                st_ = strip.tile([P, C], BF16, name="strip")
                nc.vector.tensor_scalar_mul(
                    out=st_[:], in0=ps.rearrange("p a b -> p (a b)"),
                    scalar1=rws_sb[:])
                nc.sync.dma_start(
                    out=rs_in[g][tg // 2, (tg % 2) * P:(tg % 2 + 1) * P, :],
                    in_=st_[:])

        ar_l, wt_l = {}, {}

        def gen_mid_a(g, tg, tag="ps"):
            """attention residual for token tile: ar DMA (tg==0, already
            token-major) + residual add into x2q. No transposes needed."""
            if tg == 0:
                ar_l[g] = midp.tile([P, QTTg, C], BF16, tag="ar", bufs=2,
                                    name="ar")
                nc.scalar.dma_start(
                    out=ar_l[g][:],
                    in_=rs_out[g].rearrange("(a p) c -> p a c", p=P))
            tl = g * QTTg + tg
            xq_tile = xs.tile([P, C], F32, name="xs")
            nc.sync.dma_start(out=xq_tile[:], in_=xq_t[tl])
            nc.vector.tensor_add(out=x2q[tl][:], in0=xq_tile[:],
                                 in1=ar_l[g][:, tg, :])
            yield

        def gen_mid_b(g, tg, tag="ps"):
            """LN2 + transpose into h2Tq[g] for token tile (g1 lands fp8)."""
            tl = g * QTTg + tg
            h2_tile = hp.tile([P, C], BF16, name="hp")
            layernorm_tile(x2q[tl], h2_tile)
            yield
            transpose_grouped(h2_tile, h2Tq[g][:, :, tg * P:(tg + 1) * P],
                              tag=tag)
            yield

        def gen_wst(g, hg, q="scalar"):
            """stream one chunk of wfc (4 hidden m-tiles); bf16 for g0,
            fp8 (the WS-prescaled copy) for g1."""
            if g == 0:
                wt = wstp.tile([P, CT, 4 * P], BF16, tag="wst", name="wst")
                src = wfc_r
            else:
                wt = wstp.tile([P, CT, 4 * P], FP8, tag="wst", name="wst8")
                src = wfc8_r
            eng = nc.scalar if q == "scalar" else nc.sync
            eng.dma_start(
                out=wt[:], in_=src[:, :, hg * 4 * P:(hg + 1) * 4 * P])
            wt_l[(g, hg)] = wt
            yield

        def gen_fc(g, hg, mp, tag="ps"):
            """two hidden m-tiles of fc for query group g; yields every ~4
            matmuls. g0: bf16; g1: fp8 DoubleRow (tail-shortening)."""
            wt = wt_l[(g, hg)]
            # [P,2,QTW] so the two halves land in different banks
            # (gelu of half j must not read the bank PE writes)
            ps = psum.tile([P, 2, QTW], F32, tag=tag,
                           bufs=(1 if tag == "fill" else None), name="ps")
            for j in range(2):
                mgl = mp * 2 + j
                mg = hg * 4 + mgl
                if g == 0:
                    for k in range(CT):
                        nc.tensor.matmul(
                            ps[:, j, 0:TS],
                            wt[:, k, mgl * P:(mgl + 1) * P],
                            h2Tq[0][:, k, :],
                            start=(k == 0), stop=(k == CT - 1))
                        if k % 4 == 3:
                            yield
                else:
                    for k2 in range(CT // 2):
                        nc.tensor.matmul(
                            ps[:, j, 0:TS],
                            wt[:, 2 * k2:2 * k2 + 2, mgl * P:(mgl + 1) * P],
                            h2Tq[1][:, 2 * k2:2 * k2 + 2, :],
                            start=(k2 == 0), stop=(k2 == CT // 2 - 1),
                            perf_mode=DR)
                    yield
                nc.scalar.activation(
                    out=fcT[g][:, mg, :], in_=ps[:, j, 0:TS],
                    func=AF.Gelu, bias=bfc_sb[:, mg:mg + 1],
                    scale=(1.0 if g == 0 else 1.0 / WS))

        def fc2_chunk(g):
            """token-major fc2 + residual + out DMA: per 128-token tile,
            psum[tok, C] accumulates fcT^T @ wfc2 over the 32 hidden k-tiles
            plus a K=1 ones-row matmul that adds b_fc2; eviction fuses the
            x2 residual add and streams straight to the output."""
            for tg in range(QTTg):
                tl = g * QTTg + tg
                ps = psum.tile([P, 2, QTW], F32, tag="ps", name="ps")
                for half in range(2):
                    c0 = half * QTW
                    for k in range(FT):
                        nc.tensor.matmul(
                            ps[:, half, :],
                            fcT[g][:, k, tg * P:(tg + 1) * P],
                            wfc2sb[:, k, c0:c0 + QTW],
                            start=(k == 0), stop=False)
                    nc.tensor.matmul(
                        ps[:, half, :], ones1[:],
                        bfc2row[:, c0:c0 + QTW],
                        start=False, stop=True)
                o_tile = xo.tile([P, C], F32, name="xo")
                nc.vector.tensor_add(out=o_tile[:], in0=x2q[tl][:],
                                     in1=ps.rearrange("p a b -> p (a b)"))
                nc.sync.dma_start(out=out_t[tl], in_=o_tile[:])

        def rs_go(g):
            nc.gpsimd.collective_compute(
                "ReduceScatter", mybir.AluOpType.add, replica_groups=GROUPS,
                ins=[rs_in[g].opt()], outs=[rs_out[g].opt()])

        def mlp_tail(g):
            """fc2 + residual + out for query group g (fc already emitted)."""
            fc2_chunk(g)

        def mlp_block(g):
            """mid + fc for query group g, emitted densely (tail path);
            first two wfc chunks prefetch before the mids."""
            run_gen(gen_wst(g, 0, q="sync"))
            run_gen(gen_wst(g, 1, q="sync"))
            for tg in range(QTTg):
                run_gen(gen_mid_a(g, tg))
                run_gen(gen_mid_b(g, tg))
            for hg in range(8):
                if hg >= 2:
                    run_gen(gen_wst(g, hg))
                for mp in range(2):
                    run_gen(gen_fc(g, hg, mp))

        if NQG == 2:
            # one rolling filler queue; each head's division gen is threaded
            # into the next head's weave so head boundaries never stall PE
            fq = []
            fstep = filler_steps(fq)
            F = "fill"

            def force(gn):
                """emit a queued gen now (before proj needs its output)."""
                if gn in fq:
                    fq.remove(gn)
                run_gen(gn)
            fq += [gen_qkv_m(0, 1, tag=F), gen_qkv_m(0, 3, tag=F)]
            dv = attention_head(0, 0, filler=fstep)
            fq += [dv, gen_qkv_m(0, 5, tag=F), gen_vaug(0, hs=(2, 3), tag=F)]
            dv = attention_head(0, 1, filler=fstep)
            fq += [dv, gen_qkv_m(1, 0, tag=F), gen_qkv_m(1, 2, tag=F)]
            dv = attention_head(0, 2, filler=fstep)
            fq += [dv, gen_qkv_m(1, 4, tag=F), gen_vaug(1, hs=(0, 1), tag=F)]
            dv = attention_head(0, 3, filler=fstep)
            fq += [dv]
            fstep.drain()
            proj_chunk(0)
            rs_go(0)
            fq += [gen_qkv_m(1, 1, tag=F), gen_qkv_m(1, 3, tag=F)]
            dv = attention_head(1, 0, filler=fstep)
            fq += [dv, gen_qkv_m(1, 5, tag=F), gen_vaug(1, hs=(2, 3), tag=F)]
            dv = attention_head(1, 1, filler=fstep)
            fstep.drain()   # all qkv fillers must emit before the pools close
            open_mlp_pools()
            fq += [dv, gen_wst(0, 0), gen_wst(0, 1),
                   gen_mid_a(0, 0, tag=F), gen_mid_b(0, 0, tag=F),
                   gen_mid_a(0, 1, tag=F), gen_mid_b(0, 1, tag=F)]
            fq += [gen_fc(0, 0, mp, tag=F) for mp in range(2)]
            fq += [gen_wst(0, 2)]
            fq += [gen_fc(0, 1, mp, tag=F) for mp in range(2)]
            fq += [gen_wst(0, 3)]
            fq += [gen_fc(0, 2, mp, tag=F) for mp in range(2)]
            fq += [gen_wst(0, 4)]
            dv = attention_head(1, 2, filler=fstep)
            fq += [dv]
            fq += [gen_fc(0, 3, mp, tag=F) for mp in range(2)]
            fq += [gen_wst(0, 5)]
            fq += [gen_fc(0, 4, mp, tag=F) for mp in range(2)]
            fq += [gen_wst(0, 6)]
            fq += [gen_fc(0, 5, mp, tag=F) for mp in range(2)]
            fq += [gen_wst(0, 7)]
            dv = attention_head(1, 3, filler=fstep)
            fq += [dv]
            fq += [gen_fc(0, 6, mp, tag=F) for mp in range(2)]
            fq += [gen_fc(0, 7, mp, tag=F) for mp in range(2)]
            fstep.drain()           # leftover fc(g0) steps + last division
            proj_chunk(1)
            rs_go(1)
            mlp_tail(0)             # fc2(g0)+res(g0)+out(g0) overlap RS_1
            mlp_block(1)
            mlp_tail(1)
        else:
            for g in range(1, NQG):
                for m in range(6):
                    run_gen(gen_qkv_m(g, m))
                run_gen(gen_vaug(g))
            open_mlp_pools()
            for g in range(NQG):
                for h in range(NH_LOC):
                    run_gen(attention_head(g, h))
                proj_chunk(g)
                rs_go(g)
            for g in range(NQG):
                mlp_block(g)
                mlp_tail(g)

    nc.compile()
    return nc


def _alibi_slopes(n_head: int) -> np.ndarray:
    def pow2_slopes(n):
        start = 2 ** (-(2 ** (-(math.log2(n) - 3))))
        return [start * start ** i for i in range(n)]
    if math.log2(n_head).is_integer():
        slopes = pow2_slopes(n_head)
    else:
        c = 2 ** math.floor(math.log2(n_head))
        slopes = pow2_slopes(c)
        extra = pow2_slopes(2 * c)[0::2]
        slopes.extend(extra[: n_head - c])
    return np.asarray(slopes, dtype=np.float32)


def make_in_maps(T, x, ln1_w, ln1_b, w_qkv, b_qkv, w_proj, b_proj,
                 ln2_w, ln2_b, w_fc, b_fc, w_fc2, b_fc2, n_head=16):
    bf = ml_dtypes.bfloat16
    f8 = ml_dtypes.float8_e4m3
    TPT = T // P
    TS = QG // 4
    slopes = _alibi_slopes(n_head)

    W1 = (ln1_w[:, None] * w_qkv).astype(np.float32)
    b1 = (b_qkv + ln1_b @ w_qkv).astype(np.float32)
    W2 = (ln2_w[:, None] * w_fc).astype(np.float32)
    b2 = (b_fc + ln2_b @ w_fc).astype(np.float32)

    wfc_full = np.ascontiguousarray(W2).astype(bf)        # shared by all cores
    wfc8_full = np.ascontiguousarray(W2 * WS).astype(f8)  # fp8 copy for g1 fc
    wfc2_full = np.ascontiguousarray(w_fc2).astype(bf)
    bfc_full = b2.reshape(4 * C // P, P).T.copy()         # [P, FT]
    bfc2_row = b_fc2.astype(bf).reshape(1, C)

    Cq = w_qkv.shape[0]
    # [128,128] upper-triangular-inclusive block: keep query >= key on the
    # diagonal tile
    mask = (np.arange(P)[None, :] >= np.arange(P)[:, None]).astype(bf)

    in_maps = []
    for c in range(NCORES):
        b, s = c // 4, c % 4
        qs = slice(256 * s, 256 * s + 256)
        wqkv_s = np.concatenate(
            [W1[:, qs], W1[:, Cq + 256 * s: Cq + 256 * s + 256],
             W1[:, 2 * Cq + 256 * s: 2 * Cq + 256 * s + 256]], axis=1)
        bqkv_s = np.concatenate(
            [b1[qs], b1[Cq + 256 * s: Cq + 256 * s + 256],
             b1[2 * Cq + 256 * s: 2 * Cq + 256 * s + 256]])
        alibi = np.zeros((P, NH_LOC * TPT), np.float32)
        for hl in range(NH_LOC):
            sl = slopes[4 * s + hl]
            for kt in range(TPT):
                alibi[:, hl * TPT + kt] = -sl * (kt * P + np.arange(P))
        # b_proj joins the residual stream exactly once per token: fold it
        # into the xq (residual/LN2 input) copy on the host
        xq_np = np.concatenate([x[b][g * QG + s * TS: g * QG + (s + 1) * TS]
                                for g in range(T // QG)], axis=0) \
            + b_proj[None, :]
        in_maps.append({
            "x": np.ascontiguousarray(x[b]).astype(bf),
            "xq": np.ascontiguousarray(xq_np, dtype=np.float32),
            "wqkv": (wqkv_s * WS).astype(f8),
            "bqkv": bqkv_s.reshape(6, P).T.copy(),
            "wproj": (np.ascontiguousarray(w_proj[qs, :]) * WS).astype(f8),
            "wfc": wfc_full,
            "wfc8": wfc8_full,
            "bfc": bfc_full,
            "wfc2": wfc2_full,
            "bfc2r": bfc2_row,
            "alibi": alibi,
            "mask": mask,
        })
    return in_maps


def assemble(results) -> np.ndarray:
    """Interleave the per-core rank-slices back into [2, T, C]."""
    TS = QG // 4
    outs = []
    for b in range(2):
        parts = [np.asarray(results[4 * b + r]["out"]) for r in range(4)]
        TQ, Cc = parts[0].shape
        T = 4 * TQ
        full = np.empty((T, Cc), parts[0].dtype)
        for g in range(T // QG):
            for r in range(4):
                full[g * QG + r * TS: g * QG + (r + 1) * TS] = \
                    parts[r][g * TS:(g + 1) * TS]
        outs.append(full)
    return np.stack(outs)


_nc_cache = {}


def kernel(**inputs) -> np.ndarray:
    inputs = {k: np.asarray(v) for k, v in inputs.items()}
    x = inputs["x"]
    B, T, _ = x.shape
    if T not in _nc_cache:
        _nc_cache[T] = _build(T)
    nc = _nc_cache[T]
    in_maps = make_in_maps(T, **inputs)
    res = run_bass_kernel_spmd(nc, in_maps, core_ids=list(range(NCORES)))
    return assemble(res.results).astype(np.float32)


if __name__ == "__main__":
    rng = np.random.default_rng(0)
    T = 2048
    ins = dict(
        x=rng.standard_normal((2, T, C), dtype=np.float32),
        ln1_w=np.ones(C, np.float32), ln1_b=np.zeros(C, np.float32),
        w_qkv=(rng.standard_normal((C, 3 * C)) * 0.02).astype(np.float32),
        b_qkv=np.zeros(3 * C, np.float32),
        w_proj=(rng.standard_normal((C, C)) * 0.02).astype(np.float32),
        b_proj=np.zeros(C, np.float32),
        ln2_w=np.ones(C, np.float32), ln2_b=np.zeros(C, np.float32),
        w_fc=(rng.standard_normal((C, 4 * C)) * 0.02).astype(np.float32),
        b_fc=np.zeros(4 * C, np.float32),
        w_fc2=(rng.standard_normal((4 * C, C)) * 0.02).astype(np.float32),
        b_fc2=np.zeros(C, np.float32),
    )
    out = kernel(**ins)
    print(out.shape, out.dtype)



# revision 65
# speedup vs baseline: 1.1747x; 1.1747x over previous
"""Trainium2 Bass kernel for nn_AlibiBlock (dense transformer block with ALiBi).

Contract: kernel(**inputs) takes the FULL unsharded inputs (numpy or jax,
shapes from setup_inputs) and returns the FULL [2, 2048, 1024] float32 output.

Sharding (8 NeuronCores = 2 groups of 4):
  - data parallel over batch (B=2): cores 0-3 <- batch 0, cores 4-7 <- batch 1
  - tensor parallel over heads inside each group for attention (16 heads -> 4
    per core); one grouped ReduceScatter per query group hands each core the
    summed residual update for its OWN T-slice.
  - the MLP runs T-parallel (each core: its T-quarter with the FULL 4096
    hidden, weights streamed), split by query group so each half overlaps a
    ReduceScatter of the other half.

v4 structure (vs v2 baseline):
  - startup: bias consts load as single batched [P,n] DMAs (v2 issued 54
    per-column DMAs, serializing ~50us of sync-queue issue ahead of the x
    tiles); warm-collective DMAs moved to the gpsimd queue; mask/alibi on
    the scalar queue; the 8MB resident wfc2 load is emitted after the x
    tiles in sync-queue order.
  - qkv and proj run as fp8e4m3 DoubleRow matmuls (weights pre-scaled by
    WS=64 on the host, descaled by 1/WS in the evictions; h^T and the
    divided attention output yd are stored fp8). Measured rel err ~6.8e-3
    vs the 2e-2 gate (fp8 for the MLP matmuls was tried and rejected: each
    fp8 tensor-point there costs ~1.2e-2).
  - proj is computed token-major (out[token, C]: yd is the stationary
    operand, wproj the moving one), so the ReduceScatter buffers are
    [rank, TS, C], each proj strip is ONE contiguous DMA, the post-RS
    residual needs NO PE transposes, and b_proj folds into the host-side
    xq copy.
  - fc2 is also token-major: psum[token, C] accumulates fcT^T @ wfc2 over
    the 32 hidden k-tiles plus a K=1 ones-row matmul adding b_fc2; the
    eviction fuses the x2 residual add and streams straight out. wfc2 is
    SBUF-resident bf16 (loaded once); wfc streams in 1MB chunks.
  - group 1's fc (the post-RS1 tail) runs fp8 DoubleRow from a second,
    WS-prescaled fp8 copy of wfc (h2T(g1) stored fp8); group 0's fc stays
    bf16 since it is woven under attention g1 for free. Measured rel err
    1.39e-2. fp8 RS wire was tried: same speed, worse margin - rejected.
  - x is passed pre-cast to bf16 (LN1 is its only consumer; the residual
    path uses the fp32 xq), halving the startup DMA and doubling the DVE
    rate of the LN1 stats/apply; the x2 residual tiles are bf16 too, and
    the freed SBUF deepens the xs/ptp pipelines. Final: 485972 ns at rel
    err 1.40e-2.

Per-core dataflow (T=2048, C=1024, 4 heads of d=64, fp32 PSUM
accumulation, bf16 residual stream):
  LN1 via bn_stats in [T,C] layout; PE-transpose -> h^T [C,T] fp8;
  qkv^T = Wqkv^T @ h^T (DoubleRow, 1/WS+bias fused in DVE eviction), LN
  tiles 8-15 woven between the qkv column chunks; v transposed into vaug
  with a ones column (row 64 of the PV psum = softmax denominator);
  per query group and head: S^T tile = k^T @ q^T narrowed to columns >=
  kt*P-qcol; P^T = Exp(S/sqrt(d) - slope*k) in one narrowed ACT op;
  causal mask = bf16 multiply on the [128,128] diagonal block only; PV
  accumulates V_aug^T @ P^T; denominator divide via reciprocal + PE
  ones-row broadcast + DVE multiply into yd fp8;
  proj (token-major DoubleRow) -> grouped ReduceScatter (bf16);
  then per query group: residual + LN2 -> h2^T -> fc (gelu+bias in ACT
  eviction) -> token-major fc2 + residual -> out slice; the g=0 MLP is
  woven under attention g1 / RS_0, fc2(g0) overlaps RS_1.

LN affine params are folded into the qkv/fc weights on the host.
"""

import math
import sys

for _p in ("/opt/trn_rl_repo",):
    if _p not in sys.path:
        sys.path.insert(0, _p)

import numpy as np
import ml_dtypes

import concourse.bass as bass
import concourse.mybir as mybir
import concourse.tile as tile
from concourse import bacc
from concourse.bass_utils import run_bass_kernel_spmd
from concourse.masks import make_identity

BF16 = mybir.dt.bfloat16
FP8 = mybir.dt.float8e4
F32 = mybir.dt.float32
AF = mybir.ActivationFunctionType
DR = mybir.MatmulPerfMode.DoubleRow
WS = 64.0            # fp8 weight pre-scale for the MLP weights

C = 1024            # model dim
NH_LOC = 4          # heads per core
D = 64              # head dim
EPS = 1e-5
NCORES = 8
GROUPS = [[0, 1, 2, 3], [4, 5, 6, 7]]
P = 128
QTW = 512           # matmul free-dim tile (one PSUM bank)
QG = 1024           # query group / pipeline chunk width


def _build(T: int):
    """Build + compile the SPMD program for sequence length T (multiple of QG)."""
    TPT = T // P        # token partition-tiles
    CT = C // P         # 8
    NQG = T // QG       # query-group chunks
    TQ = T // 4         # T-quarter owned by each core after ReduceScatter
    QTT = TQ // P       # local token tiles
    TS = QG // 4        # local tokens contributed by one query group
    QTTg = TS // P      # local token tiles per query group
    FT = 4 * C // P     # 32 hidden partition-tiles (full MLP hidden)

    nc = bacc.Bacc("TRN2", target_bir_lowering=False, debug=False,
                   num_devices=NCORES)

    x_d = nc.dram_tensor("x", [T, C], BF16, kind="ExternalInput")
    xq_d = nc.dram_tensor("xq", [TQ, C], F32, kind="ExternalInput")
    wqkv_d = nc.dram_tensor("wqkv", [C, 3 * NH_LOC * D], FP8, kind="ExternalInput")
    bqkv_d = nc.dram_tensor("bqkv", [P, 6], F32, kind="ExternalInput")
    wproj_d = nc.dram_tensor("wproj", [NH_LOC * D, C], FP8, kind="ExternalInput")
    wfc_d = nc.dram_tensor("wfc", [C, 4 * C], BF16, kind="ExternalInput")
    wfc8_d = nc.dram_tensor("wfc8", [C, 4 * C], FP8, kind="ExternalInput")
    bfc_d = nc.dram_tensor("bfc", [P, FT], F32, kind="ExternalInput")
    wfc2_d = nc.dram_tensor("wfc2", [4 * C, C], BF16, kind="ExternalInput")
    bfc2_d = nc.dram_tensor("bfc2r", [1, C], BF16, kind="ExternalInput")
    alibi_d = nc.dram_tensor("alibi", [P, NH_LOC * TPT], F32, kind="ExternalInput")
    mask_d = nc.dram_tensor("mask", [P, P], BF16, kind="ExternalInput")
    out_d = nc.dram_tensor("out", [TQ, C], F32, kind="ExternalOutput")

    x_t = x_d.ap().rearrange("(n p) c -> n p c", p=P)
    xq_t = xq_d.ap().rearrange("(n p) c -> n p c", p=P)
    out_t = out_d.ap().rearrange("(n p) c -> n p c", p=P)
    wqkv_r = wqkv_d.ap().rearrange("(k p) m -> p k m", p=P)  # [P, CT, 768]
    wproj_r = wproj_d.ap().rearrange("(k p) m -> p k m", p=P)  # [P, 2, C]
    wfc_r = wfc_d.ap().rearrange("(k p) m -> p k m", p=P)    # [P, CT, 4C]
    wfc8_r = wfc8_d.ap().rearrange("(k p) m -> p k m", p=P)  # [P, CT, 4C] fp8
    wfc2_r = wfc2_d.ap().rearrange("(k p) m -> p k m", p=P)  # [P, FT, C]

    import contextlib

    with tile.TileContext(nc) as tc, contextlib.ExitStack() as es:
        const = es.enter_context(tc.tile_pool(name="const", bufs=1))
        wproj_pool = es.enter_context(tc.tile_pool(name="wprojp", bufs=1))
        dram = es.enter_context(tc.tile_pool(name="dram", bufs=1, space="DRAM"))
        psum = es.enter_context(tc.tile_pool(name="psum", bufs=2, space="PSUM"))
        xs = es.enter_context(tc.tile_pool(name="xs", bufs=3))
        hp = es.enter_context(tc.tile_pool(name="hp", bufs=2))
        lnp = es.enter_context(tc.tile_pool(name="lnp", bufs=4))
        strip = es.enter_context(tc.tile_pool(name="strip", bufs=2))
        xo = es.enter_context(tc.tile_pool(name="xo", bufs=1))
        apool = es.enter_context(tc.tile_pool(name="apool", bufs=1))
        ptp = es.enter_context(tc.tile_pool(name="ptp", bufs=5))
        rbp = es.enter_context(tc.tile_pool(name="rbp", bufs=1))

        # ---- constants ----
        # small consts go first on the sync queue as SINGLE batched DMAs (the
        # v2 kernel issued 54 per-column DMAs here, serializing ~50us of sync
        # queue issue ahead of the first x tile); the big resident MLP weights
        # stream on the scalar/gpsimd queues so they never block the x tiles.
        ident = const.tile([P, P], BF16)
        make_identity(nc, ident)
        mask_sb = const.tile([P, P], BF16)
        nc.scalar.dma_start(out=mask_sb[:], in_=mask_d.ap())
        alibi_sb = const.tile([P, NH_LOC * TPT], F32)
        nc.scalar.dma_start(out=alibi_sb[:], in_=alibi_d.ap())
        eps_sb = const.tile([P, 1], F32)
        nc.vector.memset(eps_sb[:], EPS)
        ones64 = const.tile([1, D], F32)
        nc.vector.memset(ones64[:], 1.0)
        rws_sb = const.tile([P, 1], F32)
        nc.vector.memset(rws_sb[:], 1.0 / WS)
        ones1 = const.tile([1, P], BF16)
        nc.vector.memset(ones1[:], 1.0)
        bqkv_sb = const.tile([P, 6], F32)
        nc.sync.dma_start(out=bqkv_sb[:], in_=bqkv_d.ap())
        bfc_sb = const.tile([P, FT], F32)
        nc.scalar.dma_start(out=bfc_sb[:], in_=bfc_d.ap())
        bfc2row = const.tile([1, C], BF16)
        nc.scalar.dma_start(out=bfc2row[:], in_=bfc2_d.ap())

        # fp8 DoubleRow attention weights (pre-scaled by WS on the host)
        wproj8 = const.tile([P, 2, C], FP8)
        nc.scalar.dma_start(out=wproj8[:], in_=wproj_r[:])

        # resident bf16 fc2 weights [P, ktile, C]; the DMA is emitted late
        # (after the LN1 x tiles) so its 8MB never starves the x loads
        wfc2sb = const.tile([P, FT, C], BF16)

        def load_wfc2():
            for h in range(2):
                nc.sync.dma_start(out=wfc2sb[:, 16 * h:16 * (h + 1), :],
                                  in_=wfc2_r[:, 16 * h:16 * (h + 1), :])

        # Per-query-group ReduceScatter buffers; each core owns rank-slice
        # cols [r*TS, (r+1)*TS) of every query group.
        # token-major proj partials: shard r = rank r's token slice [TS, C]
        rs_in = [dram.tile([4, TS, C], BF16, name=f"rs_in{g}")
                 for g in range(NQG)]
        rs_out = [dram.tile([TS, C], BF16, name=f"rs_out{g}")
                  for g in range(NQG)]

        warm_in = dram.tile([4, 1, P], BF16, name="warm_in")
        warm_out = dram.tile([1, P], BF16, name="warm_out")
        zrow = const.tile([1, P], BF16)
        nc.gpsimd.memset(zrow[:], 0.0)
        for r in range(4):
            nc.gpsimd.dma_start(out=warm_in[r], in_=zrow[:])
        nc.gpsimd.collective_compute(
            "ReduceScatter", mybir.AluOpType.add, replica_groups=GROUPS,
            ins=[warm_in.opt()], outs=[warm_out.opt()])

        def layernorm_tile(x_tile, h_tile):
            """h = (x - mean)/sqrt(var+eps), bf16 out. x [P, C] fp32."""
            st = lnp.tile([P, 2, 6], F32, name="st")
            xr = x_tile.rearrange("p (a b) -> p a b", a=2)
            for a in range(2):
                nc.vector.bn_stats(out=st[:, a, :], in_=xr[:, a, :])
            mv = lnp.tile([P, 2], F32, name="mv")
            nc.vector.bn_aggr(out=mv[:], in_=st[:])
            rs = lnp.tile([P, 1], F32, name="rs")
            nc.scalar.activation(out=rs[:], in_=mv[:, 1:2], func=AF.Sqrt,
                                 bias=eps_sb[:], scale=1.0)
            nc.vector.reciprocal(out=rs[:], in_=rs[:])
            nc.vector.tensor_scalar(out=h_tile[:], in0=x_tile[:],
                                    scalar1=mv[:, 0:1], scalar2=rs[:],
                                    op0=mybir.AluOpType.subtract,
                                    op1=mybir.AluOpType.mult)

        def transpose_grouped(h_tile, dst, tag="ps", on_act=False):
            """h [P(tok), C] -> dst [P, CT, P] column block (one evict)."""
            tp = psum.tile([P, CT, P], BF16, tag=tag,
                           bufs=(1 if tag == "fill" else None), name="tp")
            for j in range(CT):
                nc.tensor.transpose(tp[:, j, :], h_tile[:, j * P:(j + 1) * P],
                                    ident[:])
            if on_act:
                nc.scalar.activation(out=dst, in_=tp[:], func=AF.Copy)
            else:
                nc.vector.tensor_copy(out=dst, in_=tp[:])

        # ---- scoped pools: qkv weights + h^T live only through qkv ----
        es_qkv = contextlib.ExitStack()
        wqkv_pool = es_qkv.enter_context(tc.tile_pool(name="wqkvp", bufs=1))
        hTp = es_qkv.enter_context(tc.tile_pool(name="hTp", bufs=1))
        wqkv8 = wqkv_pool.tile([P, CT, 3 * NH_LOC * D], FP8, name="wqkv8")
        nc.sync.dma_start(out=wqkv8[:], in_=wqkv_r[:])
        hT = hTp.tile([P, CT, T], FP8, name="hT")

        qkvT = apool.tile([P, 6, T], BF16, name="qkvT")
        vaug = apool.tile([P, NH_LOC, TPT, D + 1], BF16, name="vaug")
        yd = apool.tile([P, 2, T], FP8, name="yd")

        def ln_tile(tt):
            # x arrives pre-cast to bf16 (LN1 is the only consumer of the
            # full x; the residual path uses the fp32 xq) -> half the DMA
            # bytes and 2x DVE rate on bn_stats/apply; hT evict on ACT
            x_tile = xs.tile([P, C], BF16, tag="xs", name="xsb")
            nc.sync.dma_start(out=x_tile[:], in_=x_t[tt])
            h_tile = hp.tile([P, C], BF16, name="hp")
            layernorm_tile(x_tile, h_tile)
            transpose_grouped(h_tile, hT[:, :, tt * P:(tt + 1) * P],
                              on_act=True)

        def gen_qkv_m(g, m, tag="ps"):
            """generator: one qkv output strip (fp8 DoubleRow); yields every
            256-col sub-block so it can weave between attention kts."""
            ps = psum.tile([P, 2, QTW], F32, tag=tag,
                           bufs=(1 if tag == "fill" else None), name="ps")
            for half in range(2):
                col = g * QG + half * QTW
                for c2 in range(2):
                    for k2 in range(CT // 2):
                        nc.tensor.matmul(
                            ps[:, half, c2 * 256:(c2 + 1) * 256],
                            wqkv8[:, 2 * k2:2 * k2 + 2, m * P:(m + 1) * P],
                            hT[:, 2 * k2:2 * k2 + 2,
                               col + c2 * 256:col + (c2 + 1) * 256],
                            start=(k2 == 0), stop=(k2 == CT // 2 - 1),
                            perf_mode=DR)
                    yield
            nc.vector.tensor_scalar(
                out=qkvT[:, m, g * QG:(g + 1) * QG],
                in0=ps.rearrange("p a b -> p (a b)"),
                scalar1=rws_sb[:], scalar2=bqkv_sb[:, m:m + 1],
                op0=mybir.AluOpType.mult, op1=mybir.AluOpType.add)

        def gen_vaug(g, hs=None, tag="ps"):
            kts = range(g * 8, min((g + 1) * 8, TPT))
            for h in (range(NH_LOC) if hs is None else hs):
                voff = (h % 2) * D
                tpv = psum.tile([P, len(kts), D], BF16, tag=tag,
                                bufs=(1 if tag == "fill" else None), name="tpv")
                for i, kt in enumerate(kts):
                    nc.tensor.transpose(
                        tpv[:, i, :],
                        qkvT[voff:voff + D, 4 + h // 2, kt * P:(kt + 1) * P],
                        ident[voff:voff + D, voff:voff + D])
                nc.vector.tensor_copy(out=vaug[:, h, kts.start:kts.stop, 0:D],
                                      in_=tpv[:])
                nc.vector.memset(vaug[:, h, kts.start:kts.stop, D:D + 1], 1.0)
                yield

        def run_gen(gn):
            for _ in gn:
                pass

        def filler_steps(gens):
            """Flatten generators into a step-callable for attention weaving.
            Operates on the caller's list IN PLACE so work appended later is
            picked up."""

            def step(n=1):
                for _ in range(n):
                    while gens:
                        try:
                            next(gens[0])
                            break
                        except StopIteration:
                            gens.pop(0)

            def drain():
                while gens:
                    run_gen(gens.pop(0))

            step.drain = drain
            return step

        # ---------- LN1 + qkv + vaug ----------
        # qkv(0) strip steps weave between the LN tiles so the PE is dense
        # from the start: half0 steps (token tiles 0-3) may run from tile 4,
        # half1 steps (tiles 4-7) from tile 8.
        def gen_qkv_half(m, half):
            """one 512-wide half of a g0 qkv strip (fp8 DoubleRow); own
            1-bank psum + per-half eviction so the fill slot releases."""
            ps = psum.tile([P, QTW], F32, tag="fill", bufs=1, name="psh")
            col = half * QTW
            for c2 in range(2):
                if c2 == 1:
                    yield
                for k2 in range(CT // 2):
                    nc.tensor.matmul(
                        ps[:, c2 * 256:(c2 + 1) * 256],
                        wqkv8[:, 2 * k2:2 * k2 + 2, m * P:(m + 1) * P],
                        hT[:, 2 * k2:2 * k2 + 2,
                           col + c2 * 256:col + (c2 + 1) * 256],
                        start=(k2 == 0), stop=(k2 == CT // 2 - 1),
                        perf_mode=DR)
            nc.vector.tensor_scalar(
                out=qkvT[:, m, col:col + QTW], in0=ps[:],
                scalar1=rws_sb[:], scalar2=bqkv_sb[:, m:m + 1],
                op0=mybir.AluOpType.mult, op1=mybir.AluOpType.add)

        a_ms = (0, 2, 4) if NQG == 2 else tuple(range(6))
        if NQG == 2 and TPT == 16:
            # halves sequenced so half1 (token tiles 4-7) only emits after
            # ln_tile(7); each half-gen fully releases before the next starts
            order = [gen_qkv_half(m, h) for h, pair in
                     ((0, (0, 2)), (1, (0, 2)), (0, (4,)), (1, (4,)))
                     for m in pair]
            seq = [gn for gn in order for _ in range(2)]
            oi = 0
            for tt in range(TPT):
                ln_tile(tt)
                if tt >= 4 and oi < len(seq):
                    next(seq[oi], None)
                    oi += 1
            while oi < len(seq):
                next(seq[oi], None)
                oi += 1
            for gn in order:
                run_gen(gn)                          # absorb StopIterations
        else:
            for tt in range(TPT):
                ln_tile(tt)
            for m in a_ms:
                run_gen(gen_qkv_m(0, m))
        run_gen(gen_vaug(0, hs=((0, 1) if NQG == 2 else None)))
        load_wfc2()     # 8MB resident load, after the x tiles in queue order

        # mid / MLP pools open after the last qkv use so they can reuse the
        # qkv-weight + h^T SBUF space; variables bound here, filled by
        # open_mlp_pools() at the right program point.
        midp = fcp = wstp = None
        x2q = h2Tq = None
        fcT = {}

        def open_mlp_pools():
            nonlocal midp, fcp, wstp, x2q, h2Tq
            es_qkv.close()
            midp = es.enter_context(tc.tile_pool(name="midp", bufs=1))
            # bufs=1: fcT(g1) writes begin only after fc2(g0) consumed fcT(g0)
            fcp = es.enter_context(tc.tile_pool(name="fcp", bufs=1))
            wstp = es.enter_context(tc.tile_pool(name="wstp", bufs=2))
            x2q = [midp.tile([P, C], BF16, name=f"x2q{t}")
                   for t in range(QTT)]
            # per-group h2^T: g0 bf16 (fc in bf16, woven under attention g1),
            # g1 fp8 (fc in fp8 DoubleRow to shrink the post-RS1 tail)
            h2Tq = [midp.tile([P, CT, TS], BF16, name="h2Tq0"),
                    midp.tile([P, CT, TS], FP8, name="h2Tq1")]
            for g in range(NQG):
                fcT[g] = fcp.tile([P, FT, TS], BF16, tag="fcT", name="fcT")

        def attention_head(g, h, filler=None, prologue=None):
            """One head, kt chains pipelined (scores of kt+1 interleave the PV
            of kt); score, exp and PV narrowed to the causal staircase.
            `filler` is a step-callable popped once per kt so the PE stays
            dense while ACT streams the exps."""
            qcol = g * QG
            nkt = 8 * g + 8
            off = (h % 2) * D
            qT = qkvT[off:off + D, h // 2, :]
            kT = qkvT[off:off + D, 2 + h // 2, :]
            if prologue is not None:
                run_gen(prologue)
            yps = psum.tile([D + 1, 2, QTW], F32, tag="yps", bufs=1,
                            name="yps")

            def emit_pv(kt, r, pt):
                last = (kt == nkt - 1)
                if r < QTW:
                    nc.tensor.matmul(
                        yps[:, 0, r:QTW], vaug[:, h, kt, :], pt[:, r:QTW],
                        start=(kt == 0), stop=(kt == 8 * g + 3))
                    nc.tensor.matmul(
                        yps[:, 1, :], vaug[:, h, kt, :], pt[:, QTW:QG],
                        start=(kt == 0), stop=last)
                else:
                    nc.tensor.matmul(
                        yps[:, 1, r - QTW:], vaug[:, h, kt, :],
                        pt[:, r:QG], start=False, stop=last)

            pending = None
            for kt in range(nkt):
                rr = kt * P - qcol
                r = max(rr, 0)
                bias_ap = alibi_sb[:, h * TPT + kt:h * TPT + kt + 1]
                sps = psum.tile([P, 2, QTW], F32, tag="ps", name="sps")
                if r < QTW:
                    nc.tensor.matmul(
                        sps[:, 0, r:QTW], kT[:, kt * P:(kt + 1) * P],
                        qT[:, qcol + r:qcol + QTW], start=True, stop=True)
                    nc.tensor.matmul(
                        sps[:, 1, :], kT[:, kt * P:(kt + 1) * P],
                        qT[:, qcol + QTW:qcol + QG], start=True, stop=True)
                else:
                    nc.tensor.matmul(
                        sps[:, 1, r - QTW:], kT[:, kt * P:(kt + 1) * P],
                        qT[:, qcol + r:qcol + QG], start=True, stop=True)
                pt = ptp.tile([P, QG], BF16, name="pt")
                nc.scalar.activation(
                    out=pt[:, r:], in_=sps.rearrange("p a b -> p (a b)")[:, r:],
                    func=AF.Exp, bias=bias_ap, scale=1.0 / math.sqrt(D))
                if rr >= 0:
                    nc.vector.tensor_mul(
                        out=pt[:, r:r + P], in0=pt[:, r:r + P],
                        in1=mask_sb[:])
                if pending is not None:
                    emit_pv(*pending)
                pending = (kt, r, pt)
                if filler is not None:
                    filler()
            if pending is not None:
                emit_pv(*pending)

            def gen_division():
                dn = rbp.tile([1, QG], F32, name="dn")
                nc.vector.tensor_copy(
                    out=dn[:],
                    in_=yps[D:D + 1, :, :].rearrange("p a b -> p (a b)"))
                nc.vector.reciprocal_approx_fast(out=dn[:], in_=dn[:])
                # broadcast 1/dn over D partitions on the PE (K=1 matmul with
                # a ones row) so gpsimd stays free for collectives
                rbps = psum.tile([D, 2, QTW], F32, tag="fill", bufs=1,
                                 name="rbps")
                for half in range(2):
                    nc.tensor.matmul(rbps[:, half, :], ones64[:],
                                     dn[:, half * QTW:(half + 1) * QTW],
                                     start=True, stop=True)
                rb = rbp.tile([D, QG], F32, name="rb")
                nc.vector.tensor_copy(
                    out=rb[:], in_=rbps.rearrange("p a b -> p (a b)"))
                nc.vector.tensor_mul(
                    out=yd[off:off + D, h // 2, qcol:qcol + QG],
                    in0=yps[0:D, :, :].rearrange("p a b -> p (a b)"),
                    in1=rb[:])
                yield

            return gen_division()

        def proj_chunk(g):
            """token-major proj (fp8 DoubleRow): out[token, C] per 128-token
            tile; strips go straight into the rank-sliced RS buffer with a
            single contiguous DMA each; b_proj is folded into xq on the
            host, so the eviction is just the 1/WS descale."""
            for tg in range(CT):
                tok = g * QG + tg * P
                ps = psum.tile([P, 2, QTW], F32, tag="ps", name="ps")
                for half in range(2):
                    for c2 in range(2):
                        c0 = half * QTW + c2 * 256
                        nc.tensor.matmul(
                            ps[:, half, c2 * 256:(c2 + 1) * 256],
                            yd[:, :, tok:tok + P],
                            wproj8[:, :, c0:c0 + 256],
                            start=True, stop=True, perf_mode=DR)
                st_ = strip.tile([P, C], BF16, name="strip")
                nc.vector.tensor_scalar_mul(
                    out=st_[:], in0=ps.rearrange("p a b -> p (a b)"),
                    scalar1=rws_sb[:])
                nc.sync.dma_start(
                    out=rs_in[g][tg // 2, (tg % 2) * P:(tg % 2 + 1) * P, :],
                    in_=st_[:])

        ar_l, wt_l = {}, {}

        def gen_mid_a(g, tg, tag="ps"):
            """attention residual for token tile: ar DMA (tg==0, already
            token-major) + residual add into x2q. No transposes needed."""
            if tg == 0:
                ar_l[g] = midp.tile([P, QTTg, C], BF16, tag="ar", bufs=2,
                                    name="ar")
                nc.scalar.dma_start(
                    out=ar_l[g][:],
                    in_=rs_out[g].rearrange("(a p) c -> p a c", p=P))
            tl = g * QTTg + tg
            xq_tile = xs.tile([P, C], F32, name="xs")
            nc.sync.dma_start(out=xq_tile[:], in_=xq_t[tl])
            nc.vector.tensor_add(out=x2q[tl][:], in0=xq_tile[:],
                                 in1=ar_l[g][:, tg, :])
            yield

        def gen_mid_b(g, tg, tag="ps"):
            """LN2 + transpose into h2Tq[g] for token tile (g1 lands fp8)."""
            tl = g * QTTg + tg
            h2_tile = hp.tile([P, C], BF16, name="hp")
            layernorm_tile(x2q[tl], h2_tile)
            yield
            transpose_grouped(h2_tile, h2Tq[g][:, :, tg * P:(tg + 1) * P],
                              tag=tag)
            yield

        def gen_wst(g, hg, q="scalar"):
            """stream one chunk of wfc (4 hidden m-tiles); bf16 for g0,
            fp8 (the WS-prescaled copy) for g1."""
            if g == 0:
                wt = wstp.tile([P, CT, 4 * P], BF16, tag="wst", name="wst")
                src = wfc_r
            else:
                wt = wstp.tile([P, CT, 4 * P], FP8, tag="wst", name="wst8")
                src = wfc8_r
            eng = nc.scalar if q == "scalar" else nc.sync
            eng.dma_start(
                out=wt[:], in_=src[:, :, hg * 4 * P:(hg + 1) * 4 * P])
            wt_l[(g, hg)] = wt
            yield

        def gen_fc(g, hg, mp, tag="ps"):
            """two hidden m-tiles of fc for query group g; yields every ~4
            matmuls. g0: bf16; g1: fp8 DoubleRow (tail-shortening)."""
            wt = wt_l[(g, hg)]
            # [P,2,QTW] so the two halves land in different banks
            # (gelu of half j must not read the bank PE writes)
            ps = psum.tile([P, 2, QTW], F32, tag=tag,
                           bufs=(1 if tag == "fill" else None), name="ps")
            for j in range(2):
                mgl = mp * 2 + j
                mg = hg * 4 + mgl
                if g == 0:
                    for k in range(CT):
                        nc.tensor.matmul(
                            ps[:, j, 0:TS],
                            wt[:, k, mgl * P:(mgl + 1) * P],
                            h2Tq[0][:, k, :],
                            start=(k == 0), stop=(k == CT - 1))
                        if k % 4 == 3:
                            yield
                else:
                    for k2 in range(CT // 2):
                        nc.tensor.matmul(
                            ps[:, j, 0:TS],
                            wt[:, 2 * k2:2 * k2 + 2, mgl * P:(mgl + 1) * P],
                            h2Tq[1][:, 2 * k2:2 * k2 + 2, :],
                            start=(k2 == 0), stop=(k2 == CT // 2 - 1),
                            perf_mode=DR)
                    yield
                nc.scalar.activation(
                    out=fcT[g][:, mg, :], in_=ps[:, j, 0:TS],
                    func=AF.Gelu, bias=bfc_sb[:, mg:mg + 1],
                    scale=(1.0 if g == 0 else 1.0 / WS))

        def fc2_chunk(g):
            """token-major fc2 + residual + out DMA: per 128-token tile,
            psum[tok, C] accumulates fcT^T @ wfc2 over the 32 hidden k-tiles
            plus a K=1 ones-row matmul that adds b_fc2; eviction fuses the
            x2 residual add and streams straight to the output."""
            for tg in range(QTTg):
                tl = g * QTTg + tg
                ps = psum.tile([P, 2, QTW], F32, tag="ps", name="ps")
                for half in range(2):
                    c0 = half * QTW
                    for k in range(FT):
                        nc.tensor.matmul(
                            ps[:, half, :],
                            fcT[g][:, k, tg * P:(tg + 1) * P],
                            wfc2sb[:, k, c0:c0 + QTW],
                            start=(k == 0), stop=False)
                    nc.tensor.matmul(
                        ps[:, half, :], ones1[:],
                        bfc2row[:, c0:c0 + QTW],
                        start=False, stop=True)
                o_tile = xo.tile([P, C], F32, name="xo")
                nc.vector.tensor_add(out=o_tile[:], in0=x2q[tl][:],
                                     in1=ps.rearrange("p a b -> p (a b)"))
                nc.sync.dma_start(out=out_t[tl], in_=o_tile[:])

        def rs_go(g):
            nc.gpsimd.collective_compute(
                "ReduceScatter", mybir.AluOpType.add, replica_groups=GROUPS,
                ins=[rs_in[g].opt()], outs=[rs_out[g].opt()])

        def mlp_tail(g):
            """fc2 + residual + out for query group g (fc already emitted)."""
            fc2_chunk(g)

        def mlp_block(g):
            """mid + fc for query group g, emitted densely (tail path);
            first two wfc chunks prefetch before the mids."""
            run_gen(gen_wst(g, 0, q="sync"))
            run_gen(gen_wst(g, 1, q="sync"))
            for tg in range(QTTg):
                run_gen(gen_mid_a(g, tg))
                run_gen(gen_mid_b(g, tg))
            for hg in range(8):
                if hg >= 2:
                    run_gen(gen_wst(g, hg))
                for mp in range(2):
                    run_gen(gen_fc(g, hg, mp))

        if NQG == 2:
            # one rolling filler queue; each head's division gen is threaded
            # into the next head's weave so head boundaries never stall PE
            fq = []
            fstep = filler_steps(fq)
            F = "fill"

            def force(gn):
                """emit a queued gen now (before proj needs its output)."""
                if gn in fq:
                    fq.remove(gn)
                run_gen(gn)
            fq += [gen_qkv_m(0, 1, tag=F), gen_qkv_m(0, 3, tag=F)]
            dv = attention_head(0, 0, filler=fstep)
            fq += [dv, gen_qkv_m(0, 5, tag=F), gen_vaug(0, hs=(2, 3), tag=F)]
            dv = attention_head(0, 1, filler=fstep)
            fq += [dv, gen_qkv_m(1, 0, tag=F), gen_qkv_m(1, 2, tag=F)]
            dv = attention_head(0, 2, filler=fstep)
            fq += [dv, gen_qkv_m(1, 4, tag=F), gen_vaug(1, hs=(0, 1), tag=F)]
            dv = attention_head(0, 3, filler=fstep)
            fq += [dv]
            fstep.drain()
            proj_chunk(0)
            rs_go(0)
            fq += [gen_qkv_m(1, 1, tag=F), gen_qkv_m(1, 3, tag=F)]
            dv = attention_head(1, 0, filler=fstep)
            fq += [dv, gen_qkv_m(1, 5, tag=F), gen_vaug(1, hs=(2, 3), tag=F)]
            dv = attention_head(1, 1, filler=fstep)
            fstep.drain()   # all qkv fillers must emit before the pools close
            open_mlp_pools()
            fq += [dv, gen_wst(0, 0), gen_wst(0, 1),
                   gen_mid_a(0, 0, tag=F), gen_mid_b(0, 0, tag=F),
                   gen_mid_a(0, 1, tag=F), gen_mid_b(0, 1, tag=F)]
            fq += [gen_fc(0, 0, mp, tag=F) for mp in range(2)]
            fq += [gen_wst(0, 2)]
            fq += [gen_fc(0, 1, mp, tag=F) for mp in range(2)]
            fq += [gen_wst(0, 3)]
            fq += [gen_fc(0, 2, mp, tag=F) for mp in range(2)]
            fq += [gen_wst(0, 4)]
            dv = attention_head(1, 2, filler=fstep)
            fq += [dv]
            fq += [gen_fc(0, 3, mp, tag=F) for mp in range(2)]
            fq += [gen_wst(0, 5)]
            fq += [gen_fc(0, 4, mp, tag=F) for mp in range(2)]
            fq += [gen_wst(0, 6)]
            fq += [gen_fc(0, 5, mp, tag=F) for mp in range(2)]
            fq += [gen_wst(0, 7)]
            dv = attention_head(1, 3, filler=fstep)
            fq += [dv]
            fq += [gen_fc(0, 6, mp, tag=F) for mp in range(2)]
            fq += [gen_fc(0, 7, mp, tag=F) for mp in range(2)]
            fstep.drain()           # leftover fc(g0) steps + last division
            proj_chunk(1)
            rs_go(1)
            mlp_tail(0)             # fc2(g0)+res(g0)+out(g0) overlap RS_1
            mlp_block(1)
            mlp_tail(1)
        else:
            for g in range(1, NQG):
                for m in range(6):
                    run_gen(gen_qkv_m(g, m))
                run_gen(gen_vaug(g))
            open_mlp_pools()
            for g in range(NQG):
                for h in range(NH_LOC):
                    run_gen(attention_head(g, h))
                proj_chunk(g)
                rs_go(g)
            for g in range(NQG):
                mlp_block(g)
                mlp_tail(g)

    nc.compile()
    return nc


def _alibi_slopes(n_head: int) -> np.ndarray:
    def pow2_slopes(n):
        start = 2 ** (-(2 ** (-(math.log2(n) - 3))))
        return [start * start ** i for i in range(n)]
    if math.log2(n_head).is_integer():
        slopes = pow2_slopes(n_head)
    else:
        c = 2 ** math.floor(math.log2(n_head))
        slopes = pow2_slopes(c)
        extra = pow2_slopes(2 * c)[0::2]
        slopes.extend(extra[: n_head - c])
    return np.asarray(slopes, dtype=np.float32)


def make_in_maps(T, x, ln1_w, ln1_b, w_qkv, b_qkv, w_proj, b_proj,
                 ln2_w, ln2_b, w_fc, b_fc, w_fc2, b_fc2, n_head=16):
    bf = ml_dtypes.bfloat16
    f8 = ml_dtypes.float8_e4m3
    TPT = T // P
    TS = QG // 4
    slopes = _alibi_slopes(n_head)

    W1 = (ln1_w[:, None] * w_qkv).astype(np.float32)
    b1 = (b_qkv + ln1_b @ w_qkv).astype(np.float32)
    W2 = (ln2_w[:, None] * w_fc).astype(np.float32)
    b2 = (b_fc + ln2_b @ w_fc).astype(np.float32)

    wfc_full = np.ascontiguousarray(W2).astype(bf)        # shared by all cores
    wfc8_full = np.ascontiguousarray(W2 * WS).astype(f8)  # fp8 copy for g1 fc
    wfc2_full = np.ascontiguousarray(w_fc2).astype(bf)
    bfc_full = b2.reshape(4 * C // P, P).T.copy()         # [P, FT]
    bfc2_row = b_fc2.astype(bf).reshape(1, C)

    Cq = w_qkv.shape[0]
    # [128,128] upper-triangular-inclusive block: keep query >= key on the
    # diagonal tile
    mask = (np.arange(P)[None, :] >= np.arange(P)[:, None]).astype(bf)

    in_maps = []
    for c in range(NCORES):
        b, s = c // 4, c % 4
        qs = slice(256 * s, 256 * s + 256)
        wqkv_s = np.concatenate(
            [W1[:, qs], W1[:, Cq + 256 * s: Cq + 256 * s + 256],
             W1[:, 2 * Cq + 256 * s: 2 * Cq + 256 * s + 256]], axis=1)
        bqkv_s = np.concatenate(
            [b1[qs], b1[Cq + 256 * s: Cq + 256 * s + 256],
             b1[2 * Cq + 256 * s: 2 * Cq + 256 * s + 256]])
        alibi = np.zeros((P, NH_LOC * TPT), np.float32)
        for hl in range(NH_LOC):
            sl = slopes[4 * s + hl]
            for kt in range(TPT):
                alibi[:, hl * TPT + kt] = -sl * (kt * P + np.arange(P))
        # b_proj joins the residual stream exactly once per token: fold it
        # into the xq (residual/LN2 input) copy on the host
        xq_np = np.concatenate([x[b][g * QG + s * TS: g * QG + (s + 1) * TS]
                                for g in range(T // QG)], axis=0) \
            + b_proj[None, :]
        in_maps.append({
            "x": np.ascontiguousarray(x[b]).astype(bf),
            "xq": np.ascontiguousarray(xq_np, dtype=np.float32),
            "wqkv": (wqkv_s * WS).astype(f8),
            "bqkv": bqkv_s.reshape(6, P).T.copy(),
            "wproj": (np.ascontiguousarray(w_proj[qs, :]) * WS).astype(f8),
            "wfc": wfc_full,
            "wfc8": wfc8_full,
            "bfc": bfc_full,
            "wfc2": wfc2_full,
            "bfc2r": bfc2_row,
            "alibi": alibi,
            "mask": mask,
        })
    return in_maps


def assemble(results) -> np.ndarray:
    """Interleave the per-core rank-slices back into [2, T, C]."""
    TS = QG // 4
    outs = []
    for b in range(2):
        parts = [np.asarray(results[4 * b + r]["out"]) for r in range(4)]
        TQ, Cc = parts[0].shape
        T = 4 * TQ
        full = np.empty((T, Cc), parts[0].dtype)
        for g in range(T // QG):
            for r in range(4):
                full[g * QG + r * TS: g * QG + (r + 1) * TS] = \
                    parts[r][g * TS:(g + 1) * TS]
        outs.append(full)
    return np.stack(outs)


_nc_cache = {}


def kernel(**inputs) -> np.ndarray:
    inputs = {k: np.asarray(v) for k, v in inputs.items()}
    x = inputs["x"]
    B, T, _ = x.shape
    if T not in _nc_cache:
        _nc_cache[T] = _build(T)
    nc = _nc_cache[T]
    in_maps = make_in_maps(T, **inputs)
    res = run_bass_kernel_spmd(nc, in_maps, core_ids=list(range(NCORES)))
    return assemble(res.results).astype(np.float32)


if __name__ == "__main__":
    rng = np.random.default_rng(0)
    T = 2048
    ins = dict(
        x=rng.standard_normal((2, T, C), dtype=np.float32),
        ln1_w=np.ones(C, np.float32), ln1_b=np.zeros(C, np.float32),
        w_qkv=(rng.standard_normal((C, 3 * C)) * 0.02).astype(np.float32),
        b_qkv=np.zeros(3 * C, np.float32),
        w_proj=(rng.standard_normal((C, C)) * 0.02).astype(np.float32),
        b_proj=np.zeros(C, np.float32),
        ln2_w=np.ones(C, np.float32), ln2_b=np.zeros(C, np.float32),
        w_fc=(rng.standard_normal((C, 4 * C)) * 0.02).astype(np.float32),
        b_fc=np.zeros(4 * C, np.float32),
        w_fc2=(rng.standard_normal((4 * C, C)) * 0.02).astype(np.float32),
        b_fc2=np.zeros(C, np.float32),
    )
    out = kernel(**ins)
    print(out.shape, out.dtype)

